# revision 16
# baseline (speedup 1.0000x reference)
"""Sparse (masked, cosine-similarity) attention kernel for Trainium2.

Problem: B=8, LQ=LK=2048, D=64.
  q_n = q / ||q||, k_n = k / ||k||
  S = q_n @ k_n^T           (per batch)
  S = where(mask==0, -1e9, S)
  P = softmax(S, axis=-1)
  out = P @ V
  returns (out, P)

Strategy (data-parallel over batch; one batch row per NeuronCore, 8 cores):
  - Scores are cosine similarities in [-1, 1], so softmax needs no
    max-subtraction: exp(S) is in [e^-1, e^1]. Masked entries become
    exp(S)*mask = 0 exactly (mask is 0/1).
  - Per 128-row q-tile:
      S tile     = PE matmul (fp32), qT/kT in [D, L] layout (PE-transposed once)
      E          = exp(S) on ScalarE, PSUM->SBUF, bf16
      P_un, Z    = (E * mask, row-sum) in ONE VectorE tensor_tensor_reduce
      p_attn out = P_un * (1/Z) on VectorE (fp32 out) -> DMA
      P_un^T     = DMA xbar transpose (bf16, 128x128 blocks)
      out        = accumulate PE matmuls P_un^T(j).T @ V(j) -> * (1/Z) -> DMA
"""

import os
import numpy as np
from contextlib import ExitStack

import concourse.bass as bass
import concourse.bacc as bacc
import concourse.mybir as mybir
import concourse.tile as tile
from concourse.bass_utils import run_bass_kernel_spmd
from concourse.masks import make_identity

B, LQ, LK, D = 8, 2048, 2048, 64
P = 128
NQT, NKT = LQ // P, LK // P
KCH = 512            # scores free-dim chunk (one PSUM bank of fp32)
NCH = LK // KCH
N_CORES = 8

f32 = mybir.dt.float32
f32r = mybir.dt.float32r
bf16 = mybir.dt.bfloat16
i32 = mybir.dt.int32
AX = mybir.AxisListType
OP = mybir.AluOpType
AF = mybir.ActivationFunctionType


def _normalize_and_transpose(tc, prep, prep_ps, x_sb, dstT, ident, nt):
    """Rows of x_sb[:, t, :] ([P, D] tiles) -> L2-normalized, transposed into
    dstT ([D, L], d on partitions)."""
    nc = tc.nc
    for t in range(nt):
        sq = prep.tile([P, D], f32, tag="sq")
        nc.vector.tensor_mul(sq, x_sb[:, t, :], x_sb[:, t, :])
        ssum = prep.tile([P, 1], f32, tag="ssum")
        nc.vector.tensor_reduce(ssum, sq, axis=AX.X, op=OP.add)
        nrm = prep.tile([P, 1], f32, tag="nrm")
        nc.scalar.sqrt(nrm, ssum)
        inv = prep.tile([P, 1], f32, tag="inv")
        nc.vector.reciprocal(inv, nrm)
        xn = prep.tile([P, D], f32, tag="xn")
        nc.vector.tensor_scalar_mul(xn, x_sb[:, t, :], inv)
        ps = prep_ps.tile([P, P], f32, tag="tps")
        nc.tensor.transpose(ps[:D, :], xn, ident)
        nc.scalar.copy(dstT[:, t * P:(t + 1) * P], ps[:D, :])


def _attention(ctx: ExitStack, tc: tile.TileContext, q, k, v, msk, out, p_attn):
    nc = tc.nc

    const = ctx.enter_context(tc.tile_pool(name="const", bufs=1))
    ident = const.tile([P, P], f32)
    make_identity(nc, ident)
    qT = const.tile([D, LQ], f32r)
    kT = const.tile([D, LK], f32r)
    v_bf = const.tile([P, NKT, D], bf16)

    # ---- phase 0: load q/k/v, normalize q/k rows, build qT/kT, cast v ----
    with tc.tile_pool(name="prep", bufs=2) as prep, \
         tc.tile_pool(name="prep_ps", bufs=2, space="PSUM") as prep_ps:
        q_sb = prep.tile([P, NQT, D], f32, tag="x_sb")
        nc.gpsimd.dma_start(out=q_sb, in_=q.rearrange("(t p) d -> p t d", p=P))
        _normalize_and_transpose(tc, prep, prep_ps, q_sb, qT, ident, NQT)

        k_sb = prep.tile([P, NKT, D], f32, tag="x_sb")
        nc.gpsimd.dma_start(out=k_sb, in_=k.rearrange("(t p) d -> p t d", p=P))
        _normalize_and_transpose(tc, prep, prep_ps, k_sb, kT, ident, NKT)

        v_sb = prep.tile([P, NKT, D], f32, tag="x_sb")
        nc.gpsimd.dma_start(out=v_sb, in_=v.rearrange("(t p) d -> p t d", p=P))
        nc.vector.tensor_copy(v_bf, v_sb)

    # ---- main loop over q-tiles ----
    mpool = ctx.enter_context(tc.tile_pool(name="mpool", bufs=5))
    epool = ctx.enter_context(tc.tile_pool(name="epool", bufs=3))
    ppool = ctx.enter_context(tc.tile_pool(name="ppool", bufs=4))
    opool = ctx.enter_context(tc.tile_pool(name="opool", bufs=3))
    tpool = ctx.enter_context(tc.tile_pool(name="tpool", bufs=3))
    zpool = ctx.enter_context(tc.tile_pool(name="zpool", bufs=8))
    s_ps_pool = ctx.enter_context(tc.tile_pool(name="s_ps", bufs=6, space="PSUM"))
    av_ps_pool = ctx.enter_context(tc.tile_pool(name="av_ps", bufs=2, space="PSUM"))

    for i in range(NQT):
        # SWDGE casts int32 -> bf16 in the DMA datapath (mask is 0/1, exact).
        m_sb = mpool.tile([P, LK], bf16, tag="mask")
        nc.gpsimd.dma_start(out=m_sb, in_=msk[i * P:(i + 1) * P, :])

        E = epool.tile([P, LK], bf16, tag="E")
        for c in range(NCH):
            s_ps = s_ps_pool.tile([P, KCH], f32, tag="s")
            nc.tensor.matmul(
                s_ps,
                qT[:, i * P:(i + 1) * P],
                kT[:, c * KCH:(c + 1) * KCH],
                start=True, stop=True,
            )
            nc.scalar.activation(E[:, c * KCH:(c + 1) * KCH], s_ps, AF.Exp)

        p_un = ppool.tile([P, LK], bf16, tag="p_un")
        z = zpool.tile([P, 1], f32, tag="z")
        nc.vector.scalar_tensor_tensor(
            out=p_un, in0=E, scalar=1.0, in1=m_sb,
            op0=OP.mult, op1=OP.mult, accum_out=z,
        )
        invz = zpool.tile([P, 1], f32, tag="invz")
        nc.vector.reciprocal(invz, z)

        p_out = opool.tile([P, LK], f32, tag="p_out")
        nc.vector.tensor_scalar_mul(p_out, p_un, invz)
        nc.scalar.dma_start(out=p_attn[i * P:(i + 1) * P, :], in_=p_out)

        av_ps = av_ps_pool.tile([P, D], f32, tag="av")
        pT_all = tpool.tile([P, NKT, P], bf16, tag="pT")
        nc.sync.dma_start_transpose(out=pT_all, in_=p_un)
        for j in range(NKT):
            nc.tensor.matmul(
                av_ps, pT_all[:, j, :], v_bf[:, j, :],
                start=(j == 0), stop=(j == NKT - 1),
            )
        o_sb = zpool.tile([P, D], f32, tag="o")
        nc.vector.tensor_scalar_mul(o_sb, av_ps, invz)
        nc.scalar.dma_start(out=out[i * P:(i + 1) * P, :], in_=o_sb)


def build_graph():
    nc = bacc.Bacc("TRN2", target_bir_lowering=False, debug=False)
    q = nc.dram_tensor("query", [LQ, D], f32, kind="ExternalInput").ap()
    k = nc.dram_tensor("key", [LK, D], f32, kind="ExternalInput").ap()
    v = nc.dram_tensor("value", [LK, D], f32, kind="ExternalInput").ap()
    msk = nc.dram_tensor("mask", [LQ, LK], i32, kind="ExternalInput").ap()
    out = nc.dram_tensor("out", [LQ, D], f32, kind="ExternalOutput").ap()
    p_attn = nc.dram_tensor("p_attn", [LQ, LK], f32, kind="ExternalOutput").ap()

    with tile.TileContext(nc) as tc:
        with ExitStack() as ctx:
            _attention(ctx, tc, q, k, v, msk, out, p_attn)
    nc.compile()
    return nc


_NC_CACHE = None


def _get_graph():
    global _NC_CACHE
    if _NC_CACHE is None:
        _NC_CACHE = build_graph()
    return _NC_CACHE


def kernel(query, key, value, mask):
    query = np.asarray(query, dtype=np.float32)
    key = np.asarray(key, dtype=np.float32)
    value = np.asarray(value, dtype=np.float32)
    mask = np.asarray(mask, dtype=np.int32)

    nc = _get_graph()
    in_maps = [
        {
            "query": np.ascontiguousarray(query[b]),
            "key": np.ascontiguousarray(key[b]),
            "value": np.ascontiguousarray(value[b]),
            "mask": np.ascontiguousarray(mask[b]),
        }
        for b in range(B)
    ]
    res = run_bass_kernel_spmd(nc, in_maps, core_ids=list(range(N_CORES)))
    out = np.stack([r["out"] for r in res.results])
    p_attn = np.stack([r["p_attn"] for r in res.results])
    return out, p_attn


# revision 21
# speedup vs baseline: 2653.2966x; 2653.2966x over previous
"""Sparse (masked, cosine-similarity) attention kernel for Trainium2.

Problem: B=8, LQ=LK=2048, D=64.
  q_n = q / ||q||, k_n = k / ||k||
  S = q_n @ k_n^T           (per batch)
  S = where(mask==0, -1e9, S)
  P = softmax(S, axis=-1)
  out = P @ V
  returns (out, P)

Strategy (data-parallel over batch; one batch row per NeuronCore, 8 cores):
  - Scores are cosine similarities in [-1, 1], so softmax needs no
    max-subtraction: exp(S) is in [e^-1, e^1]. Masked entries become
    exp(S)*mask = 0 exactly (mask is 0/1).
  - Per 128-row q-tile:
      S tile     = PE matmul (fp32), qT/kT in [D, L] layout (PE-transposed once)
      E          = exp(S) on ScalarE, PSUM->SBUF, bf16
      P_un, Z    = (E * mask, row-sum) in ONE VectorE tensor_tensor_reduce
      p_attn out = P_un * (1/Z) on VectorE (fp32 out) -> DMA
      P_un^T     = DMA xbar transpose (bf16, 128x128 blocks)
      out        = accumulate PE matmuls P_un^T(j).T @ V(j) -> * (1/Z) -> DMA
"""

import os
import numpy as np
from contextlib import ExitStack

import concourse.bass as bass
import concourse.bacc as bacc
import concourse.mybir as mybir
import concourse.tile as tile
from concourse.bass_utils import run_bass_kernel_spmd
from concourse.masks import make_identity

B, LQ, LK, D = 8, 2048, 2048, 64
P = 128
NQT, NKT = LQ // P, LK // P
KCH = 512            # scores free-dim chunk (one PSUM bank of fp32)
NCH = LK // KCH
N_CORES = 8

f32 = mybir.dt.float32
f32r = mybir.dt.float32r
bf16 = mybir.dt.bfloat16
i32 = mybir.dt.int32
AX = mybir.AxisListType
OP = mybir.AluOpType
AF = mybir.ActivationFunctionType


def _normalize_and_transpose(tc, prep, prep_ps, x_sb, dstT, ident, nt):
    """Rows of x_sb[:, t, :] ([P, D] tiles) -> L2-normalized, transposed into
    dstT ([D, L], d on partitions)."""
    nc = tc.nc
    for t in range(nt):
        sq = prep.tile([P, D], f32, tag="sq")
        nc.vector.tensor_mul(sq, x_sb[:, t, :], x_sb[:, t, :])
        ssum = prep.tile([P, 1], f32, tag="ssum")
        nc.vector.tensor_reduce(ssum, sq, axis=AX.X, op=OP.add)
        nrm = prep.tile([P, 1], f32, tag="nrm")
        nc.scalar.sqrt(nrm, ssum)
        inv = prep.tile([P, 1], f32, tag="inv")
        nc.vector.reciprocal(inv, nrm)
        xn = prep.tile([P, D], f32, tag="xn")
        nc.vector.tensor_scalar_mul(xn, x_sb[:, t, :], inv)
        ps = prep_ps.tile([P, P], f32, tag="tps")
        nc.tensor.transpose(ps[:D, :], xn, ident)
        nc.scalar.copy(dstT[:, t * P:(t + 1) * P], ps[:D, :])


def _attention(ctx: ExitStack, tc: tile.TileContext, q, k, v, msk, out, p_attn,
               sfx=""):
    nc = tc.nc

    const = ctx.enter_context(tc.tile_pool(name=f"const{sfx}", bufs=1))
    ident = const.tile([P, P], f32)
    make_identity(nc, ident)
    qT = const.tile([D, LQ], f32r)
    kT = const.tile([D, LK], f32r)
    v_bf = const.tile([P, NKT, D], bf16)

    # ---- phase 0: load q/k/v, normalize q/k rows, build qT/kT, cast v ----
    with tc.tile_pool(name=f"prep{sfx}", bufs=2) as prep, \
         tc.tile_pool(name=f"prep_ps{sfx}", bufs=2, space="PSUM") as prep_ps:
        q_sb = prep.tile([P, NQT, D], f32, tag="x_sb")
        nc.gpsimd.dma_start(out=q_sb, in_=q.rearrange("(t p) d -> p t d", p=P))
        _normalize_and_transpose(tc, prep, prep_ps, q_sb, qT, ident, NQT)

        k_sb = prep.tile([P, NKT, D], f32, tag="x_sb")
        nc.gpsimd.dma_start(out=k_sb, in_=k.rearrange("(t p) d -> p t d", p=P))
        _normalize_and_transpose(tc, prep, prep_ps, k_sb, kT, ident, NKT)

        v_sb = prep.tile([P, NKT, D], f32, tag="x_sb")
        nc.gpsimd.dma_start(out=v_sb, in_=v.rearrange("(t p) d -> p t d", p=P))
        nc.vector.tensor_copy(v_bf, v_sb)

    # ---- main loop over q-tiles ----
    mpool = ctx.enter_context(tc.tile_pool(name=f"mpool{sfx}", bufs=5))
    epool = ctx.enter_context(tc.tile_pool(name=f"epool{sfx}", bufs=3))
    ppool = ctx.enter_context(tc.tile_pool(name=f"ppool{sfx}", bufs=4))
    opool = ctx.enter_context(tc.tile_pool(name=f"opool{sfx}", bufs=3))
    tpool = ctx.enter_context(tc.tile_pool(name=f"tpool{sfx}", bufs=3))
    zpool = ctx.enter_context(tc.tile_pool(name=f"zpool{sfx}", bufs=8))
    s_ps_pool = ctx.enter_context(tc.tile_pool(name=f"s_ps{sfx}", bufs=6, space="PSUM"))
    av_ps_pool = ctx.enter_context(tc.tile_pool(name=f"av_ps{sfx}", bufs=2, space="PSUM"))

    for i in range(NQT):
        # SWDGE casts int32 -> bf16 in the DMA datapath (mask is 0/1, exact).
        m_sb = mpool.tile([P, LK], bf16, tag="mask")
        nc.gpsimd.dma_start(out=m_sb, in_=msk[i * P:(i + 1) * P, :])

        E = epool.tile([P, LK], bf16, tag="E")
        for c in range(NCH):
            s_ps = s_ps_pool.tile([P, KCH], f32, tag="s")
            nc.tensor.matmul(
                s_ps,
                qT[:, i * P:(i + 1) * P],
                kT[:, c * KCH:(c + 1) * KCH],
                start=True, stop=True,
            )
            nc.scalar.activation(E[:, c * KCH:(c + 1) * KCH], s_ps, AF.Exp)

        p_un = ppool.tile([P, LK], bf16, tag="p_un")
        z = zpool.tile([P, 1], f32, tag="z")
        nc.vector.scalar_tensor_tensor(
            out=p_un, in0=E, scalar=1.0, in1=m_sb,
            op0=OP.mult, op1=OP.mult, accum_out=z,
        )
        invz = zpool.tile([P, 1], f32, tag="invz")
        nc.vector.reciprocal(invz, z)

        p_out = opool.tile([P, LK], f32, tag="p_out")
        nc.vector.tensor_scalar_mul(p_out, p_un, invz)
        nc.scalar.dma_start(out=p_attn[i * P:(i + 1) * P, :], in_=p_out)

        av_ps = av_ps_pool.tile([P, D], f32, tag="av")
        pT_all = tpool.tile([P, NKT, P], bf16, tag="pT")
        nc.sync.dma_start_transpose(out=pT_all, in_=p_un)
        for j in range(NKT):
            nc.tensor.matmul(
                av_ps, pT_all[:, j, :], v_bf[:, j, :],
                start=(j == 0), stop=(j == NKT - 1),
            )
        o_sb = zpool.tile([P, D], f32, tag="o")
        nc.vector.tensor_scalar_mul(o_sb, av_ps, invz)
        nc.scalar.dma_start(out=out[i * P:(i + 1) * P, :], in_=o_sb)


def build_graph(reps=1):
    nc = bacc.Bacc("TRN2", target_bir_lowering=False, debug=False)
    q = nc.dram_tensor("query", [LQ, D], f32, kind="ExternalInput").ap()
    k = nc.dram_tensor("key", [LK, D], f32, kind="ExternalInput").ap()
    v = nc.dram_tensor("value", [LK, D], f32, kind="ExternalInput").ap()
    msk = nc.dram_tensor("mask", [LQ, LK], i32, kind="ExternalInput").ap()
    out = nc.dram_tensor("out", [LQ, D], f32, kind="ExternalOutput").ap()
    p_attn = nc.dram_tensor("p_attn", [LQ, LK], f32, kind="ExternalOutput").ap()

    with tile.TileContext(nc) as tc:
        for r in range(reps):
            with ExitStack() as ctx:
                _attention(ctx, tc, q, k, v, msk, out, p_attn,
                           sfx=f"_{r}" if reps > 1 else "")
    nc.compile()
    return nc


_NC_CACHE = None


def _get_graph():
    global _NC_CACHE
    if _NC_CACHE is None:
        _NC_CACHE = build_graph()
    return _NC_CACHE


_RUNNER_CACHE = None

IN_NAMES = ["query", "key", "value", "mask"]
OUT_NAMES = ["out", "p_attn"]


def _get_runner():
    """Cached jitted SPMD executable (one NEFF on each of the 8 cores)."""
    global _RUNNER_CACHE
    if _RUNNER_CACHE is not None:
        return _RUNNER_CACHE

    import jax
    from jax.sharding import Mesh, PartitionSpec
    from jax.experimental.shard_map import shard_map
    from concourse import bass2jax
    from concourse.bass2jax import _bass_exec_p

    bass2jax.install_neuronx_cc_hook()
    nc = _get_graph()

    out_avals = [
        jax.core.ShapedArray((LQ, D), np.float32),
        jax.core.ShapedArray((LQ, LK), np.float32),
    ]
    # outputs passed as zero buffers; partition_id supplied last (PartitionIdOp)
    in_names = IN_NAMES + OUT_NAMES
    if nc.partition_id_tensor is not None:
        in_names = in_names + [nc.partition_id_tensor.name]

    def _body(*args):
        operands = list(args)
        if nc.partition_id_tensor is not None:
            operands.append(bass2jax.partition_id_tensor())
        outs = _bass_exec_p.bind(
            *operands,
            out_avals=tuple(out_avals),
            in_names=tuple(in_names),
            out_names=tuple(OUT_NAMES),
            lowering_input_output_aliases=(),
            sim_require_finite=True,
            sim_require_nnan=True,
            nc=nc,
        )
        return tuple(outs)

    devices = jax.devices()[:N_CORES]
    mesh = Mesh(np.asarray(devices), ("core",))
    in_specs = (PartitionSpec("core"),) * 6
    out_specs = (PartitionSpec("core"),) * 2
    sharded = jax.jit(
        shard_map(_body, mesh=mesh, in_specs=in_specs, out_specs=out_specs,
                  check_rep=False),
        keep_unused=True,
    )
    _RUNNER_CACHE = (sharded, mesh)
    return _RUNNER_CACHE


def kernel(query, key, value, mask):
    query = np.ascontiguousarray(np.asarray(query, dtype=np.float32)).reshape(B * LQ, D)
    key = np.ascontiguousarray(np.asarray(key, dtype=np.float32)).reshape(B * LK, D)
    value = np.ascontiguousarray(np.asarray(value, dtype=np.float32)).reshape(B * LK, D)
    mask = np.ascontiguousarray(np.asarray(mask, dtype=np.int32)).reshape(B * LQ, LK)

    sharded, _ = _get_runner()
    zo = np.zeros((B * LQ, D), np.float32)
    zp = np.zeros((B * LQ, LK), np.float32)
    out_c, p_c = sharded(query, key, value, mask, zo, zp)
    out = np.asarray(out_c).reshape(B, LQ, D)
    p_attn = np.asarray(p_c).reshape(B, LQ, LK)
    return out, p_attn


# revision 29
# speedup vs baseline: 8513.2417x; 3.2086x over previous
"""Sparse (masked, cosine-similarity) attention kernel for Trainium2.

Problem: B=8, LQ=LK=2048, D=64.
  q_n = q / ||q||, k_n = k / ||k||
  S = q_n @ k_n^T           (per batch)
  S = where(mask==0, -1e9, S)
  P = softmax(S, axis=-1)
  out = P @ V
  returns (out, P)

Strategy (data-parallel over batch; one batch row per NeuronCore, 8 cores):
  - Scores are cosine similarities in [-1, 1], so softmax needs no
    max-subtraction: exp(S) is in [e^-1, e^1]. Masked entries become
    exp(S)*mask = 0 exactly (mask is 0/1).
  - Per 128-row q-tile:
      S tile     = PE matmul (fp32), qT/kT in [D, L] layout (PE-transposed once)
      E          = exp(S) on ScalarE, PSUM->SBUF, bf16
      P_un, Z    = (E * mask, row-sum) in ONE VectorE tensor_tensor_reduce
      p_attn out = P_un * (1/Z) on VectorE (fp32 out) -> DMA
      P_un^T     = DMA xbar transpose (bf16, 128x128 blocks)
      out        = accumulate PE matmuls P_un^T(j).T @ V(j) -> * (1/Z) -> DMA
"""

import os
import numpy as np
from contextlib import ExitStack

import concourse.bass as bass
import concourse.bacc as bacc
import concourse.mybir as mybir
import concourse.tile as tile
from concourse.bass_utils import run_bass_kernel_spmd
from concourse.masks import make_identity

B, LQ, LK, D = 8, 2048, 2048, 64
P = 128
NQT, NKT = LQ // P, LK // P
KCH = 512            # scores free-dim chunk (one PSUM bank of fp32)
NCH = LK // KCH
N_CORES = 8

f32 = mybir.dt.float32
f32r = mybir.dt.float32r
bf16 = mybir.dt.bfloat16
i32 = mybir.dt.int32
AX = mybir.AxisListType
OP = mybir.AluOpType
AF = mybir.ActivationFunctionType


def _normalize_and_transpose(tc, prep, prep_ps, x_sb, dstT, ident, nt):
    """Rows of x_sb[:, t, :] ([P, D] tiles) -> L2-normalized, transposed into
    dstT ([D, L], d on partitions)."""
    nc = tc.nc
    for t in range(nt):
        sq = prep.tile([P, D], f32, tag="sq")
        nc.vector.tensor_mul(sq, x_sb[:, t, :], x_sb[:, t, :])
        ssum = prep.tile([P, 1], f32, tag="ssum")
        nc.vector.tensor_reduce(ssum, sq, axis=AX.X, op=OP.add)
        nrm = prep.tile([P, 1], f32, tag="nrm")
        nc.scalar.sqrt(nrm, ssum)
        inv = prep.tile([P, 1], f32, tag="inv")
        nc.vector.reciprocal(inv, nrm)
        xn = prep.tile([P, D], f32, tag="xn")
        nc.vector.tensor_scalar_mul(xn, x_sb[:, t, :], inv)
        ps = prep_ps.tile([P, P], f32, tag="tps")
        nc.tensor.transpose(ps[:D, :], xn, ident)
        nc.scalar.copy(dstT[:, t * P:(t + 1) * P], ps[:D, :])


def _attention(ctx: ExitStack, tc: tile.TileContext, q, k, v, msk, out, p_attn,
               sfx=""):
    nc = tc.nc

    const = ctx.enter_context(tc.tile_pool(name=f"const{sfx}", bufs=1))
    ident = const.tile([P, P], f32)
    make_identity(nc, ident)
    qT = const.tile([D, LQ], f32r)
    kT = const.tile([D, LK], f32r)
    v_bf = const.tile([P, NKT, D], bf16)

    # ---- phase 0: load q/k/v, normalize q/k rows, build qT/kT, cast v ----
    with tc.tile_pool(name=f"prep{sfx}", bufs=2) as prep, \
         tc.tile_pool(name=f"prep_ps{sfx}", bufs=2, space="PSUM") as prep_ps:
        q_sb = prep.tile([P, NQT, D], f32, tag="x_sb")
        nc.gpsimd.dma_start(out=q_sb, in_=q.rearrange("(t p) d -> p t d", p=P))
        _normalize_and_transpose(tc, prep, prep_ps, q_sb, qT, ident, NQT)

        k_sb = prep.tile([P, NKT, D], f32, tag="x_sb")
        nc.gpsimd.dma_start(out=k_sb, in_=k.rearrange("(t p) d -> p t d", p=P))
        _normalize_and_transpose(tc, prep, prep_ps, k_sb, kT, ident, NKT)

        v_sb = prep.tile([P, NKT, D], f32, tag="x_sb")
        nc.gpsimd.dma_start(out=v_sb, in_=v.rearrange("(t p) d -> p t d", p=P))
        nc.vector.tensor_copy(v_bf, v_sb)

    # ---- main loop over q-tiles ----
    mpool = ctx.enter_context(tc.tile_pool(name=f"mpool{sfx}", bufs=8))
    epool = ctx.enter_context(tc.tile_pool(name=f"epool{sfx}", bufs=4))
    ppool = ctx.enter_context(tc.tile_pool(name=f"ppool{sfx}", bufs=5))
    opool = ctx.enter_context(tc.tile_pool(name=f"opool{sfx}", bufs=4))
    tpool = ctx.enter_context(tc.tile_pool(name=f"tpool{sfx}", bufs=4))
    zpool = ctx.enter_context(tc.tile_pool(name=f"zpool{sfx}", bufs=8))
    s_ps_pool = ctx.enter_context(tc.tile_pool(name=f"s_ps{sfx}", bufs=6, space="PSUM"))
    av_ps_pool = ctx.enter_context(tc.tile_pool(name=f"av_ps{sfx}", bufs=2, space="PSUM"))

    for i in range(NQT):
        # SWDGE casts int32 -> bf16 in the DMA datapath (mask is 0/1, exact).
        m_sb = mpool.tile([P, LK], bf16, tag="mask")
        nc.gpsimd.dma_start(out=m_sb, in_=msk[i * P:(i + 1) * P, :])

        E = epool.tile([P, LK], bf16, tag="E")
        for c in range(NCH):
            s_ps = s_ps_pool.tile([P, KCH], f32, tag="s")
            nc.tensor.matmul(
                s_ps,
                qT[:, i * P:(i + 1) * P],
                kT[:, c * KCH:(c + 1) * KCH],
                start=True, stop=True,
            )
            nc.scalar.activation(E[:, c * KCH:(c + 1) * KCH], s_ps, AF.Exp)

        p_un = ppool.tile([P, LK], bf16, tag="p_un")
        z = zpool.tile([P, 1], f32, tag="z")
        nc.vector.scalar_tensor_tensor(
            out=p_un, in0=E, scalar=1.0, in1=m_sb,
            op0=OP.mult, op1=OP.mult, accum_out=z,
        )
        invz = zpool.tile([P, 1], f32, tag="invz")
        nc.vector.reciprocal(invz, z)

        p_out = opool.tile([P, LK], f32, tag="p_out")
        nc.vector.tensor_scalar_mul(p_out, p_un, invz)
        peng = nc.scalar if i % 2 == 0 else nc.sync
        peng.dma_start(out=p_attn[i * P:(i + 1) * P, :], in_=p_out)

        av_ps = av_ps_pool.tile([P, D], f32, tag="av")
        pT_all = tpool.tile([P, NKT, P], bf16, tag="pT")
        nc.sync.dma_start_transpose(out=pT_all, in_=p_un)
        for j in range(NKT):
            nc.tensor.matmul(
                av_ps, pT_all[:, j, :], v_bf[:, j, :],
                start=(j == 0), stop=(j == NKT - 1),
            )
        o_sb = zpool.tile([P, D], f32, tag="o")
        nc.vector.tensor_scalar_mul(o_sb, av_ps, invz)
        nc.scalar.dma_start(out=out[i * P:(i + 1) * P, :], in_=o_sb)


def build_graph(reps=1):
    nc = bacc.Bacc("TRN2", target_bir_lowering=False, debug=False)
    q = nc.dram_tensor("query", [LQ, D], f32, kind="ExternalInput").ap()
    k = nc.dram_tensor("key", [LK, D], f32, kind="ExternalInput").ap()
    v = nc.dram_tensor("value", [LK, D], f32, kind="ExternalInput").ap()
    msk = nc.dram_tensor("mask", [LQ, LK], i32, kind="ExternalInput").ap()
    out = nc.dram_tensor("out", [LQ, D], f32, kind="ExternalOutput").ap()
    p_attn = nc.dram_tensor("p_attn", [LQ, LK], f32, kind="ExternalOutput").ap()

    with tile.TileContext(nc) as tc:
        for r in range(reps):
            with ExitStack() as ctx:
                _attention(ctx, tc, q, k, v, msk, out, p_attn,
                           sfx=f"_{r}" if reps > 1 else "")
    nc.compile()
    return nc


_NC_CACHE = None


def _get_graph():
    global _NC_CACHE
    if _NC_CACHE is None:
        _NC_CACHE = build_graph()
    return _NC_CACHE


_RUNNER_CACHE = None

IN_NAMES = ["query", "key", "value", "mask"]
OUT_NAMES = ["out", "p_attn"]


def _get_runner():
    """Cached jitted SPMD executable (one NEFF on each of the 8 cores)."""
    global _RUNNER_CACHE
    if _RUNNER_CACHE is not None:
        return _RUNNER_CACHE

    import jax
    from jax.sharding import Mesh, PartitionSpec
    from jax.experimental.shard_map import shard_map
    from concourse import bass2jax
    from concourse.bass2jax import _bass_exec_p

    bass2jax.install_neuronx_cc_hook()
    nc = _get_graph()

    out_avals = [
        jax.core.ShapedArray((LQ, D), np.float32),
        jax.core.ShapedArray((LQ, LK), np.float32),
    ]
    # outputs passed as zero buffers; partition_id supplied last (PartitionIdOp)
    in_names = IN_NAMES + OUT_NAMES
    if nc.partition_id_tensor is not None:
        in_names = in_names + [nc.partition_id_tensor.name]

    def _body(*args):
        operands = list(args)
        if nc.partition_id_tensor is not None:
            operands.append(bass2jax.partition_id_tensor())
        outs = _bass_exec_p.bind(
            *operands,
            out_avals=tuple(out_avals),
            in_names=tuple(in_names),
            out_names=tuple(OUT_NAMES),
            lowering_input_output_aliases=(),
            sim_require_finite=True,
            sim_require_nnan=True,
            nc=nc,
        )
        return tuple(outs)

    devices = jax.devices()[:N_CORES]
    mesh = Mesh(np.asarray(devices), ("core",))
    in_specs = (PartitionSpec("core"),) * 6
    out_specs = (PartitionSpec("core"),) * 2
    sharded = jax.jit(
        shard_map(_body, mesh=mesh, in_specs=in_specs, out_specs=out_specs,
                  check_rep=False),
        keep_unused=True,
    )
    from jax.sharding import NamedSharding
    sh = NamedSharding(mesh, PartitionSpec("core"))
    zo = jax.device_put(np.zeros((B * LQ, D), np.float32), sh)
    zp = jax.device_put(np.zeros((B * LQ, LK), np.float32), sh)
    _RUNNER_CACHE = (sharded, mesh, zo, zp)
    return _RUNNER_CACHE


def kernel(query, key, value, mask):
    query = np.ascontiguousarray(np.asarray(query, dtype=np.float32)).reshape(B * LQ, D)
    key = np.ascontiguousarray(np.asarray(key, dtype=np.float32)).reshape(B * LK, D)
    value = np.ascontiguousarray(np.asarray(value, dtype=np.float32)).reshape(B * LK, D)
    mask = np.ascontiguousarray(np.asarray(mask, dtype=np.int32)).reshape(B * LQ, LK)

    sharded, _, zo, zp = _get_runner()
    out_c, p_c = sharded(query, key, value, mask, zo, zp)
    out = np.asarray(out_c).reshape(B, LQ, D)
    p_attn = np.asarray(p_c).reshape(B, LQ, LK)
    return out, p_attn


# revision 47
# speedup vs baseline: 39151.8022x; 4.5989x over previous
"""Sparse (masked, cosine-similarity) attention kernel for Trainium2.

Problem: B=8, LQ=LK=2048, D=64.
  q_n = q / ||q||, k_n = k / ||k||
  S = q_n @ k_n^T           (per batch)
  S = where(mask==0, -1e9, S)
  P = softmax(S, axis=-1)
  out = P @ V
  returns (out, P)

Strategy (data-parallel over batch; one batch row per NeuronCore, 8 cores):
  - Scores are cosine similarities in [-1, 1], so softmax needs no
    max-subtraction: exp(S) is in [e^-1, e^1]. Masked entries become
    exp(S)*mask = 0 exactly (mask is 0/1).
  - Per 128-row q-tile:
      S chunks   = PE matmuls in float32r (1 cyc/row at N=512; qT/kT built
                   once via PE transposes, one tile each so deps are fine)
      E          = exp(S) on ScalarE, PSUM->SBUF, bf16
      mask       = SWDGE DMA with int32->bf16 cast in the DMA datapath
      P_un, Z    = (E * mask, row-sum) in ONE VectorE scalar_tensor_tensor
                   (tensor_tensor_reduce crashes this HW revision)
      p_attn     = P_un * (1/Z) on VectorE -> 1MB DMA (rings alternate)
      P_un^T     = one DMA xbar transpose per tile ([128,2048] -> 16 blocks)
      out        = 16 accumulating PE matmuls P_un^T(j).T @ V(j), * (1/Z),
                   gathered into one [128,16,64] tile, single DMA at the end
  - DMA queue split: mask loads on SWDGE (first 8 in an ungated warm pool to
    fill the prep-phase DMA idle), transposes on SP HWDGE, p-stores
    alternating SP/ACT HWDGE rings -> no head-of-line blocking anywhere.
"""

import os
import numpy as np
from contextlib import ExitStack

import concourse.bass as bass
import concourse.bacc as bacc
import concourse.mybir as mybir
import concourse.tile as tile
from concourse.bass_utils import run_bass_kernel_spmd
from concourse.masks import make_identity

B, LQ, LK, D = 8, 2048, 2048, 64
P = 128
NQT, NKT = LQ // P, LK // P
KCH = 512            # scores free-dim chunk (one PSUM bank of fp32)
NCH = LK // KCH
N_CORES = 8

f32 = mybir.dt.float32
f32r = mybir.dt.float32r
bf16 = mybir.dt.bfloat16
i32 = mybir.dt.int32
AX = mybir.AxisListType
OP = mybir.AluOpType
AF = mybir.ActivationFunctionType


def _normalize_and_transpose(tc, prep, prep_ps, x_sb, dstT, ident, nt):
    """Rows of x_sb[:, t, :] ([P, D] tiles) -> L2-normalized, transposed into
    dstT ([D, L], d on partitions)."""
    nc = tc.nc
    G = 4
    for t in range(nt):
        sq = prep.tile([P, D], f32, tag="sq")
        nc.vector.tensor_mul(sq, x_sb[:, t, :], x_sb[:, t, :])
        ssum = prep.tile([P, 1], f32, tag="ssum")
        nc.vector.tensor_reduce(ssum, sq, axis=AX.X, op=OP.add)
        nrm = prep.tile([P, 1], f32, tag="nrm")
        nc.scalar.sqrt(nrm, ssum)
        inv = prep.tile([P, 1], f32, tag="inv")
        nc.vector.reciprocal(inv, nrm)
        xn = prep.tile([P, D], f32, tag="xn")
        nc.vector.tensor_scalar_mul(xn, x_sb[:, t, :], inv)
        ps = prep_ps.tile([P, P], f32, tag="tps")
        nc.tensor.transpose(ps[:D, :], xn, ident)
        nc.scalar.copy(dstT(t), ps[:D, :])


def _attention(ctx: ExitStack, tc: tile.TileContext, q, k, v, msk, out, p_attn,
               sfx=""):
    nc = tc.nc

    const = ctx.enter_context(tc.tile_pool(name=f"const{sfx}", bufs=1))
    ident = const.tile([P, P], f32)
    make_identity(nc, ident)
    # per-q-tile / per-chunk tiles so the first S matmuls depend only on the
    # slices they read, not on the whole prep phase
    qT_tiles = []
    for t in range(NQT):
        qTt = const.tile([D, P], f32r, tag=f"qTt{t}")
        qT_tiles.append(qTt)
    kT_chunks = []
    for cix in range(NCH):
        kTc = const.tile([D, KCH], f32r, tag=f"kTc{cix}")
        kT_chunks.append(kTc)
    v_bf = const.tile([P, NKT, D], bf16)

    # ---- phase 0: load q/k/v, normalize q/k rows, build qT/kT, cast v ----
    with tc.tile_pool(name=f"prep{sfx}", bufs=6) as prep, \
         tc.tile_pool(name=f"prep_ps{sfx}", bufs=2, space="PSUM") as prep_ps:
        k_sb = prep.tile([P, NKT, D], f32, tag="x_sb")
        nc.gpsimd.dma_start(out=k_sb, in_=k.rearrange("(t p) d -> p t d", p=P))
        _normalize_and_transpose(tc, prep, prep_ps, k_sb,
                                 lambda t: kT_chunks[t // 4][:, (t % 4) * P:(t % 4 + 1) * P],
                                 ident, NKT)

        q_sb = prep.tile([P, NQT, D], f32, tag="x_sb")
        nc.gpsimd.dma_start(out=q_sb, in_=q.rearrange("(t p) d -> p t d", p=P))
        _normalize_and_transpose(tc, prep, prep_ps, q_sb,
                                 lambda t: qT_tiles[t][:, :], ident, NQT)

        v_sb = prep.tile([P, NKT, D], f32, tag="x_sb")
        nc.gpsimd.dma_start(out=v_sb, in_=v.rearrange("(t p) d -> p t d", p=P))
        nc.vector.tensor_copy(v_bf, v_sb)

    # ---- main loop over q-tiles ----
    mpool = ctx.enter_context(tc.tile_pool(name=f"mpool{sfx}", bufs=7))
    epool = ctx.enter_context(tc.tile_pool(name=f"epool{sfx}", bufs=4))
    ppool = ctx.enter_context(tc.tile_pool(name=f"ppool{sfx}", bufs=5))
    opool = ctx.enter_context(tc.tile_pool(name=f"opool{sfx}", bufs=4))
    tpool = ctx.enter_context(tc.tile_pool(name=f"tpool{sfx}", bufs=4))
    zpool = ctx.enter_context(tc.tile_pool(name=f"zpool{sfx}", bufs=8))
    s_ps_pool = ctx.enter_context(tc.tile_pool(name=f"s_ps{sfx}", bufs=6, space="PSUM"))
    av_ps_pool = ctx.enter_context(tc.tile_pool(name=f"av_ps{sfx}", bufs=2, space="PSUM"))

    o_all = const.tile([P, NQT, D], f32)
    for i in range(NQT):
        # SWDGE casts int32 -> bf16 in the DMA datapath (mask is 0/1, exact).
        mp = mwarm if i < 8 else mpool
        m_sb = mp.tile([P, LK], bf16, tag="mask")
        nc.gpsimd.dma_start(out=m_sb, in_=msk[i * P:(i + 1) * P, :])

        E = epool.tile([P, LK], bf16, tag="E")
        for c in range(NCH):
            s_ps = s_ps_pool.tile([P, KCH], f32, tag="s")
            nc.tensor.matmul(
                s_ps,
                qT_tiles[i][:, :],
                kT_chunks[c][:, :],
                start=True, stop=True,
            )
            nc.scalar.activation(E[:, c * KCH:(c + 1) * KCH], s_ps, AF.Exp)

        p_un = ppool.tile([P, LK], bf16, tag="p_un")
        z = zpool.tile([P, 1], f32, tag="z")
        nc.vector.scalar_tensor_tensor(
            out=p_un, in0=E, scalar=1.0, in1=m_sb,
            op0=OP.mult, op1=OP.mult, accum_out=z,
        )
        invz = zpool.tile([P, 1], f32, tag="invz")
        nc.vector.reciprocal(invz, z)

        p_out = opool.tile([P, LK], f32, tag="p_out")
        nc.vector.tensor_scalar_mul(p_out, p_un, invz)
        peng = nc.scalar if i % 2 == 0 else nc.sync
        peng.dma_start(out=p_attn[i * P:(i + 1) * P, :], in_=p_out)

        av_ps = av_ps_pool.tile([P, D], f32, tag="av")
        pT_all = tpool.tile([P, NKT, P], bf16, tag="pT")
        nc.sync.dma_start_transpose(out=pT_all, in_=p_un)
        for j in range(NKT):
            nc.tensor.matmul(
                av_ps, pT_all[:, j, :], v_bf[:, j, :],
                start=(j == 0), stop=(j == NKT - 1),
            )
        nc.vector.tensor_scalar_mul(o_all[:, i, :], av_ps, invz)
        if i == NQT - 1:
            nc.scalar.dma_start(out=out.rearrange("(t p) d -> p t d", p=P), in_=o_all)


def build_graph(reps=1):
    nc = bacc.Bacc("TRN2", target_bir_lowering=False, debug=False)
    q = nc.dram_tensor("query", [LQ, D], f32, kind="ExternalInput").ap()
    k = nc.dram_tensor("key", [LK, D], f32, kind="ExternalInput").ap()
    v = nc.dram_tensor("value", [LK, D], f32, kind="ExternalInput").ap()
    msk = nc.dram_tensor("mask", [LQ, LK], i32, kind="ExternalInput").ap()
    out = nc.dram_tensor("out", [LQ, D], f32, kind="ExternalOutput").ap()
    p_attn = nc.dram_tensor("p_attn", [LQ, LK], f32, kind="ExternalOutput").ap()

    with tile.TileContext(nc) as tc:
        for r in range(reps):
            with ExitStack() as ctx:
                _attention(ctx, tc, q, k, v, msk, out, p_attn,
                           sfx=f"_{r}" if reps > 1 else "")
    nc.compile()
    return nc


_NC_CACHE = None


def _get_graph():
    global _NC_CACHE
    if _NC_CACHE is None:
        _NC_CACHE = build_graph()
    return _NC_CACHE


_RUNNER_CACHE = None

IN_NAMES = ["query", "key", "value", "mask"]
OUT_NAMES = ["out", "p_attn"]


def _get_runner():
    """Cached jitted SPMD executable (one NEFF on each of the 8 cores)."""
    global _RUNNER_CACHE
    if _RUNNER_CACHE is not None:
        return _RUNNER_CACHE

    import jax
    from jax.sharding import Mesh, PartitionSpec
    from jax.experimental.shard_map import shard_map
    from concourse import bass2jax
    from concourse.bass2jax import _bass_exec_p

    bass2jax.install_neuronx_cc_hook()
    nc = _get_graph()

    out_avals = [
        jax.core.ShapedArray((LQ, D), np.float32),
        jax.core.ShapedArray((LQ, LK), np.float32),
    ]
    # outputs passed as zero buffers; partition_id supplied last (PartitionIdOp)
    in_names = IN_NAMES + OUT_NAMES
    if nc.partition_id_tensor is not None:
        in_names = in_names + [nc.partition_id_tensor.name]

    def _body(*args):
        operands = list(args)
        if nc.partition_id_tensor is not None:
            operands.append(bass2jax.partition_id_tensor())
        outs = _bass_exec_p.bind(
            *operands,
            out_avals=tuple(out_avals),
            in_names=tuple(in_names),
            out_names=tuple(OUT_NAMES),
            lowering_input_output_aliases=(),
            sim_require_finite=True,
            sim_require_nnan=True,
            nc=nc,
        )
        return tuple(outs)

    devices = jax.devices()[:N_CORES]
    mesh = Mesh(np.asarray(devices), ("core",))
    in_specs = (PartitionSpec("core"),) * 6
    out_specs = (PartitionSpec("core"),) * 2
    sharded = jax.jit(
        shard_map(_body, mesh=mesh, in_specs=in_specs, out_specs=out_specs,
                  check_rep=False),
        keep_unused=True,
    )
    from jax.sharding import NamedSharding
    sh = NamedSharding(mesh, PartitionSpec("core"))
    zo = jax.device_put(np.zeros((B * LQ, D), np.float32), sh)
    zp = jax.device_put(np.zeros((B * LQ, LK), np.float32), sh)
    _RUNNER_CACHE = (sharded, mesh, zo, zp)
    return _RUNNER_CACHE


def kernel(query, key, value, mask):
    query = np.ascontiguousarray(np.asarray(query, dtype=np.float32)).reshape(B * LQ, D)
    key = np.ascontiguousarray(np.asarray(key, dtype=np.float32)).reshape(B * LK, D)
    value = np.ascontiguousarray(np.asarray(value, dtype=np.float32)).reshape(B * LK, D)
    mask = np.ascontiguousarray(np.asarray(mask, dtype=np.int32)).reshape(B * LQ, LK)

    sharded, _, zo, zp = _get_runner()
    out_c, p_c = sharded(query, key, value, mask, zo, zp)
    out = np.asarray(out_c).reshape(B, LQ, D)
    p_attn = np.asarray(p_c).reshape(B, LQ, LK)
    return out, p_attn
